# revision 1
# baseline (speedup 1.0000x reference)
"""GNN message passing (graph conv) on 8 Trainium2 NeuronCores.

Math:  out = elu(segment_sum(vals * (x @ W.T + b)[cols], rows))
Trick: segment_sum(v * (W x[c] + b)) = W @ segment_sum(v * x[c]) + segsum(v) * b
so we aggregate raw x rows (indirect-DMA gather + one-hot matmul) and apply
the 128x128 linear AFTER aggregation -- one matmul per 128-row output tile
instead of a [100000,128] pre-transform pass.

Sharding: destination rows are split across 8 cores (12500 each). Edges are
bucketed (host-side) by (core, dest_tile_of_128_rows), padded per-tile to a
uniform CPT chunks of 128 edges so the on-device schedule is fully static.
x is replicated to every core, so no collectives are needed.

Per core / per dest tile t (128 rows):
  G[p, c*128:+128]   = x[cols[p,c], :]          (one indirect DMA, all chunks)
  O_c[e, r]          = vals[e,c] * (iota[r] == rows[e,c])   (one tensor_scalar)
  aggT[f, r]        += G_c^T @ O_c              (PE, PSUM accumulate over c)
  z[r, o]            = aggT^T @ W^T + s[r]*b[o] (PE: lhsT=aggT, rhs=W.T; outer)
  out[t*128+r, o]    = elu(z) = max(z,0)-1 + exp(min(z,0))
"""

import numpy as np

N_NODES = 100000
D = 128
NCORES = 8
RPC = N_NODES // NCORES          # rows per core = 12500
P = 128
TILES = (RPC + P - 1) // P       # 98 dest tiles per core
RPAD = TILES * P                 # 12544 padded rows per core

# dtype config: "f32" (exact-ish) or "f16" (half gather traffic, 2x PE)
GATHER_MODE = "f16"


def _build_program(cpt: int, mode: str, debug: bool = False, reps: int = 1,
                   variant: str = "full"):
    import concourse.bass as bass
    import concourse.tile as tile
    from concourse import bacc, mybir
    from contextlib import ExitStack

    f32 = mybir.dt.float32
    i32 = mybir.dt.int32
    gdt = mybir.dt.float16 if mode == "f16" else f32

    nc = bacc.Bacc("TRN2", target_bir_lowering=False)

    x_d = nc.declare_dram_parameter("x", [N_NODES, D], gdt, isOutput=False)
    wt_d = nc.declare_dram_parameter("wT", [D, D], gdt, isOutput=False)
    b_d = nc.declare_dram_parameter("bvec", [1, D], gdt, isOutput=False)
    cols_d = nc.declare_dram_parameter("cols", [TILES, P, cpt], i32, isOutput=False)
    rows_d = nc.declare_dram_parameter("rows", [TILES, P, cpt], f32, isOutput=False)
    vals_d = nc.declare_dram_parameter("vals", [TILES, P, cpt], f32, isOutput=False)
    s_d = nc.declare_dram_parameter("svec", [TILES, 1, P], gdt, isOutput=False)
    out_d = nc.declare_dram_parameter("out", [RPAD, D], f32, isOutput=True)
    if debug:
        dbg_iota = nc.declare_dram_parameter("dbg_iota", [P, P], f32, isOutput=True)
        dbg_o = nc.declare_dram_parameter("dbg_o", [P, P], f32, isOutput=True)
        dbg_g = nc.declare_dram_parameter("dbg_g", [P, P], f32, isOutput=True)
        dbg_agg = nc.declare_dram_parameter("dbg_agg", [P, P], f32, isOutput=True)
        dbg_z = nc.declare_dram_parameter("dbg_z", [P, P], f32, isOutput=True)

    with ExitStack() as ctx:
        tc = ctx.enter_context(tile.TileContext(nc))
        const = ctx.enter_context(tc.tile_pool(name="const", bufs=1))
        edges = ctx.enter_context(tc.tile_pool(name="edges", bufs=3))
        gbuf = ctx.enter_context(tc.tile_pool(name="gbuf", bufs=3))
        ohot = ctx.enter_context(tc.tile_pool(name="ohot", bufs=6))
        work = ctx.enter_context(tc.tile_pool(name="work", bufs=3))
        psum_a = ctx.enter_context(tc.tile_pool(name="psum_a", bufs=2, space="PSUM"))
        psum_b = ctx.enter_context(tc.tile_pool(name="psum_b", bufs=2, space="PSUM"))

        wt_sb = const.tile([D, D], gdt)
        nc.sync.dma_start(wt_sb[:], wt_d[:])
        b_sb = const.tile([1, D], gdt)
        nc.sync.dma_start(b_sb[:], b_d[:])
        iota_i = const.tile([P, P], i32)
        nc.gpsimd.iota(iota_i[:], pattern=[[1, P]], base=0, channel_multiplier=0)
        iota_f = const.tile([P, P], f32)
        nc.vector.tensor_copy(iota_f[:], iota_i[:])

        rep_ctx = tc.For_i(0, reps, 1) if reps > 1 else None
        if rep_ctx is not None:
            ctx.enter_context(rep_ctx)

        for t in range(TILES):
            cols_t = edges.tile([P, cpt], i32, tag="cols")
            nc.sync.dma_start(cols_t[:], cols_d[t])
            rows_t = edges.tile([P, cpt], f32, tag="rows")
            nc.sync.dma_start(rows_t[:], rows_d[t])
            vals_t = edges.tile([P, cpt], f32, tag="vals")
            nc.sync.dma_start(vals_t[:], vals_d[t])
            s_t = edges.tile([1, P], gdt, tag="s")
            nc.sync.dma_start(s_t[:], s_d[t])

            g_t = gbuf.tile([P, cpt * P], gdt)
            # one indirect DMA per 128-edge chunk: a single multi-column
            # indirect DMA signals completion after the FIRST offset column
            # lands, racing the consuming matmuls (HW-verified), so issue
            # per-chunk gathers whose completion tracking is sound.
            if variant != "nogather":
                for c in range(cpt):
                    nc.gpsimd.indirect_dma_start(
                        out=g_t[:, c * P : (c + 1) * P],
                        out_offset=None,
                        in_=x_d[:],
                        in_offset=bass.IndirectOffsetOnAxis(
                            ap=cols_t[:, c : c + 1], axis=0
                        ),
                    )
            if variant == "onlygather":
                continue

            agg_t = psum_a.tile([P, P], f32, space="PSUM")
            for c in range(cpt):
                o_c = ohot.tile([P, P], gdt)
                eng = nc.vector
                eng.tensor_scalar(
                    out=o_c[:],
                    in0=iota_f[:],
                    scalar1=rows_t[:, c : c + 1],
                    scalar2=vals_t[:, c : c + 1],
                    op0=mybir.AluOpType.is_equal,
                    op1=mybir.AluOpType.mult,
                )
                nc.tensor.matmul(
                    agg_t[:],
                    lhsT=g_t[:, c * P : (c + 1) * P],
                    rhs=o_c[:],
                    start=(c == 0),
                    stop=(c == cpt - 1),
                )

            agg_sb = work.tile([P, P], gdt, tag="aggT")
            nc.scalar.copy(agg_sb[:], agg_t[:])
            z_t = psum_b.tile([P, P], f32, space="PSUM")
            nc.tensor.matmul(z_t[:], lhsT=agg_sb[:], rhs=wt_sb[:], start=True, stop=False)
            nc.tensor.matmul(z_t[:], lhsT=s_t[:], rhs=b_sb[:], start=False, stop=True)

            if debug and t == 0:
                dbg_sb = work.tile([P, P], f32, tag="dbg")
                nc.vector.tensor_copy(dbg_sb[:], iota_f[:])
                nc.sync.dma_start(dbg_iota[:], dbg_sb[:])
                dbg_sb2 = work.tile([P, P], f32, tag="dbg2")
                nc.vector.tensor_copy(dbg_sb2[:], o_c[:])  # last chunk's one-hot
                nc.sync.dma_start(dbg_o[:], dbg_sb2[:])
                dbg_sb3 = work.tile([P, P], f32, tag="dbg3")
                nc.vector.tensor_copy(dbg_sb3[:], g_t[:, 0:P])  # chunk 0 of G
                nc.sync.dma_start(dbg_g[:], dbg_sb3[:])
                dbg_sb4 = work.tile([P, P], f32, tag="dbg4")
                nc.vector.tensor_copy(dbg_sb4[:], agg_t[:])
                nc.sync.dma_start(dbg_agg[:], dbg_sb4[:])
                dbg_sb5 = work.tile([P, P], f32, tag="dbg5")
                nc.vector.tensor_copy(dbg_sb5[:], z_t[:])
                nc.sync.dma_start(dbg_z[:], dbg_sb5[:])

            # elu(z) = (max(z,0) - 1) + exp(min(z,0))
            zmin = work.tile([P, P], f32, tag="zmin")
            nc.vector.tensor_scalar_min(zmin[:], z_t[:], 0.0)
            pm1 = work.tile([P, P], f32, tag="pm1")
            nc.vector.tensor_scalar(
                out=pm1[:],
                in0=z_t[:],
                scalar1=0.0,
                scalar2=-1.0,
                op0=mybir.AluOpType.max,
                op1=mybir.AluOpType.add,
            )
            ez = work.tile([P, P], f32, tag="ez")
            nc.scalar.activation(ez[:], zmin[:], mybir.ActivationFunctionType.Exp)
            res = work.tile([P, P], f32, tag="res")
            nc.vector.tensor_tensor(
                out=res[:], in0=pm1[:], in1=ez[:], op=mybir.AluOpType.add
            )
            nc.scalar.dma_start(out_d[t * P : (t + 1) * P, :], res[:])

    nc.compile()
    return nc


def _prep_inputs(x, W, b, adj_rows, adj_cols, adj_vals, mode: str):
    """Host-side edge bucketing: sort by dest row, bucket into per-core
    per-tile chunk-of-128 slots with a globally uniform chunks-per-tile."""
    rows = np.ascontiguousarray(adj_rows)
    order = np.argsort(rows, kind="stable")
    r_s = rows[order]
    c_s = np.ascontiguousarray(adj_cols)[order]
    v_s = np.ascontiguousarray(adj_vals)[order].astype(np.float32)

    # rows per core = 12500 is not a multiple of 128, so the global tile id is
    # NOT simply r//128 -- compute per-core tile indices.
    core = r_s // RPC
    local = r_s - core * RPC
    tloc = local // P
    rloc = (local % P).astype(np.float32)
    gtile = core * TILES + tloc

    ntiles = NCORES * TILES
    counts = np.bincount(gtile, minlength=ntiles)
    cpt = int(np.ceil(counts.max() / P))
    slots = cpt * P

    starts = np.zeros(ntiles, dtype=np.int64)
    starts[1:] = np.cumsum(counts)[:-1]
    pos = np.arange(len(r_s), dtype=np.int64) - starts[gtile]
    dest = gtile.astype(np.int64) * slots + pos

    cols_pad = np.zeros(ntiles * slots, dtype=np.int32)
    vals_pad = np.zeros(ntiles * slots, dtype=np.float32)
    rows_pad = np.zeros(ntiles * slots, dtype=np.float32)
    cols_pad[dest] = c_s
    vals_pad[dest] = v_s
    rows_pad[dest] = rloc

    # [ntiles, cpt, P] -> [ntiles, P, cpt]: edge slot c*128+p lands at [p, c]
    def shape(a):
        return np.ascontiguousarray(a.reshape(ntiles, cpt, P).transpose(0, 2, 1))

    cols_a, vals_a, rows_a = shape(cols_pad), shape(vals_pad), shape(rows_pad)

    s_full = np.bincount(rows, weights=adj_vals.astype(np.float64), minlength=N_NODES)
    s_full = s_full.astype(np.float32)
    s_pad = np.zeros(NCORES * RPAD, dtype=np.float32)
    s_pad.reshape(NCORES, RPAD)[:, :RPC] = s_full.reshape(NCORES, RPC)
    s_a = s_pad.reshape(NCORES, TILES, 1, P)

    gnp = np.float16 if mode == "f16" else np.float32

    x_g = np.ascontiguousarray(x).astype(gnp)
    wt_g = np.ascontiguousarray(W.T).astype(gnp)
    b_g = np.ascontiguousarray(b).reshape(1, D).astype(gnp)

    in_maps = []
    for i in range(NCORES):
        in_maps.append(
            {
                "x": x_g,
                "wT": wt_g,
                "bvec": b_g,
                "cols": cols_a[i * TILES : (i + 1) * TILES],
                "rows": rows_a[i * TILES : (i + 1) * TILES],
                "vals": vals_a[i * TILES : (i + 1) * TILES],
                "svec": s_a[i].astype(gnp),
            }
        )
    return in_maps, cpt


_CACHE = {}


def _run(in_maps, cpt, mode, trace=False):
    from concourse.bass_utils import run_bass_kernel_spmd

    key = (cpt, mode)
    if key not in _CACHE:
        _CACHE[key] = _build_program(cpt, mode)
    nc = _CACHE[key]
    return run_bass_kernel_spmd(nc, in_maps, list(range(NCORES)), trace=trace)


def kernel(x, W, b, adj_rows, adj_cols, adj_vals, trace=False, _return_raw=False):
    x = np.asarray(x)
    in_maps, cpt = _prep_inputs(
        x, np.asarray(W), np.asarray(b), np.asarray(adj_rows),
        np.asarray(adj_cols), np.asarray(adj_vals), GATHER_MODE,
    )
    res = _run(in_maps, cpt, GATHER_MODE, trace=trace)
    outs = [res.results[i]["out"][:RPC] for i in range(NCORES)]
    full = np.concatenate(outs, axis=0).astype(np.float32)
    if _return_raw:
        return full, res
    return full



# revision 6
# speedup vs baseline: 3.2765x; 3.2765x over previous
"""GNN message passing (graph conv) on 8 Trainium2 NeuronCores.

Math:  out = elu(segment_sum(vals * (x @ W.T + b)[cols], rows))
Trick: segment_sum(v * (W x[c] + b)) = W @ segment_sum(v * x[c]) + segsum(v) * b
so we aggregate raw x rows (gather + one-hot matmul) and apply the 128x128
linear AFTER aggregation.

Sharding: destination rows are split across 8 cores (12500 each). Edges are
bucketed (host-side) by (core, dest_tile_of_128_rows, src_window) and padded
to 128-edge blocks. x is replicated to every core.

v3 gather: v1 used one indirect DMA per 128-edge block -- ~1.4us of Pool
engine time each (994ns SWDGE fixed overhead per instruction), 3332 of them
= 4.7ms. v3 uses the production `dma_gather` custom op (InstDMAGatherAnt):
ONE instruction gathers every block of a GT=7-tile group from one source
window (~9k rows, descriptors at 0.34ns each), with sound completion
semantics. Its int16 index limit forces splitting x into NWIN=4 windows of
25000 rows; the 4 gathers per group run on 4 separate SWDGE queues.
dma_gather layout: gathered row i lands at [i%128, i//128, :], so a block of
128 edges sharing a dest tile occupies one [128, 128] lhsT slab -- exactly
the one-hot matmul operand.

Per core / per dest tile t (128 rows):
  G[p, blk, :]  = x[idx[blk*128+p], :]           (dma_gather, 4 queues)
  O_b[e, r]     = vals[e,b] * (iota[r] == rows[e,b])   (DVE tensor_scalar)
  aggT[f, r]   += G_b^T @ O_b                    (PE, PSUM accumulate)
  z[r, o]       = aggT^T @ W^T + s[r]*b[o]
  out[t*128+r]  = elu(z) = max(z, exp(min(z,0)) - 1)
"""

import numpy as np

N_NODES = 100000
D = 128
NCORES = 8
RPC = N_NODES // NCORES          # rows per core = 12500
P = 128
TILES = (RPC + P - 1) // P       # 98 dest tiles per core
RPAD = TILES * P                 # 12544 padded rows per core
GT = 7                           # dest tiles per gather group (98 = 14*7)
NGRP = TILES // GT               # 14 groups per core
NWIN = 4
WND = N_NODES // NWIN            # 25000 rows per source window (int16 safe)


def _build_program(nblk, nblk_max):
    """nblk: [TILES][NWIN] block counts (shared across cores)."""
    import concourse.tile as tile
    from concourse import bacc, mybir
    from contextlib import ExitStack

    f32 = mybir.dt.float32
    f16 = mybir.dt.float16
    i16 = mybir.dt.int16
    i32 = mybir.dt.int32

    # per-group block layout: for w in windows: for t in group: nblk[t][w]
    # blkoff[t][w] = first block index of (t, w) within its group
    blkoff = [[0] * NWIN for _ in range(TILES)]
    gblk = [0] * NGRP                # total blocks per group
    gwblk = [[0] * NWIN for _ in range(NGRP)]  # per (group, window) blocks
    for grp in range(NGRP):
        off = 0
        for w in range(NWIN):
            for t in range(grp * GT, (grp + 1) * GT):
                blkoff[t][w] = off
                off += nblk[t][w]
                gwblk[grp][w] += nblk[t][w]
        gblk[grp] = off
    NB = nblk_max                    # uniform DRAM/tile width (max group)

    nc = bacc.Bacc("TRN2", target_bir_lowering=False, num_swdge_queues=4)

    x_d = nc.declare_dram_parameter("x", [N_NODES, D], f16, isOutput=False)
    wt_d = nc.declare_dram_parameter("wT", [D, D], f16, isOutput=False)
    b_d = nc.declare_dram_parameter("bvec", [1, D], f16, isOutput=False)
    idx_d = nc.declare_dram_parameter("idx", [NGRP, P, NB * 8], i16, isOutput=False)
    rv_d = nc.declare_dram_parameter("rv", [NGRP, P, 2 * NB], f32, isOutput=False)
    s_d = nc.declare_dram_parameter("svec", [NGRP, 1, GT * P], f16, isOutput=False)
    out_d = nc.declare_dram_parameter("out", [RPAD, D], f32, isOutput=True)

    with ExitStack() as ctx:
        tc = ctx.enter_context(tile.TileContext(nc))
        const = ctx.enter_context(tc.tile_pool(name="const", bufs=1))
        edges = ctx.enter_context(tc.tile_pool(name="edges", bufs=2))
        gbuf = ctx.enter_context(tc.tile_pool(name="gbuf", bufs=2))
        ohot = ctx.enter_context(tc.tile_pool(name="ohot", bufs=8))
        work = ctx.enter_context(tc.tile_pool(name="work", bufs=4))
        resp = ctx.enter_context(tc.tile_pool(name="resp", bufs=2))
        psum_a = ctx.enter_context(tc.tile_pool(name="psum_a", bufs=4, space="PSUM"))
        psum_b = ctx.enter_context(tc.tile_pool(name="psum_b", bufs=2, space="PSUM"))

        wt_sb = const.tile([D, D], f16)
        nc.sync.dma_start(wt_sb[:], wt_d[:])
        b_sb = const.tile([1, D], f16)
        nc.sync.dma_start(b_sb[:], b_d[:])
        iota_i = const.tile([P, P], i32)
        nc.gpsimd.iota(iota_i[:], pattern=[[1, P]], base=0, channel_multiplier=0)
        iota_f = const.tile([P, P], f16)
        nc.vector.tensor_copy(iota_f[:], iota_i[:])

        for grp in range(NGRP):
            idx_t = edges.tile([P, NB * 8], i16, tag="idx")
            nc.sync.dma_start(idx_t[:], idx_d[grp])
            rv_t = edges.tile([P, 2 * NB], f32, tag="rv")
            nc.sync.dma_start(rv_t[:], rv_d[grp])
            s_t = edges.tile([1, GT * P], f16, tag="s")
            nc.sync.dma_start(s_t[:], s_d[grp])

            g_t = gbuf.tile([P, NB, P], f16)
            off = 0
            for w in range(NWIN):
                nb_w = gwblk[grp][w]
                nc.gpsimd.dma_gather(
                    g_t[:, off : off + nb_w, :],
                    x_d[w * WND : (w + 1) * WND, :],
                    idx_t[:, off * 8 : (off + nb_w) * 8],
                    nb_w * P,
                    nb_w * P,
                    P,
                    queue_num=w,
                    single_packet=False,
                )
                off += nb_w

            res_t = resp.tile([P, GT * D], f32)
            for t in range(GT):
                tg = grp * GT + t
                blocks = []
                for w in range(NWIN):
                    b0 = blkoff[tg][w]
                    blocks.extend(range(b0, b0 + nblk[tg][w]))
                agg_t = psum_a.tile([P, P], f32, space="PSUM")
                for ci, blk in enumerate(blocks):
                    o_c = ohot.tile([P, P], f16)
                    nc.vector.tensor_scalar(
                        out=o_c[:],
                        in0=iota_f[:],
                        scalar1=rv_t[:, blk : blk + 1],
                        scalar2=rv_t[:, NB + blk : NB + blk + 1],
                        op0=mybir.AluOpType.is_equal,
                        op1=mybir.AluOpType.mult,
                    )
                    nc.tensor.matmul(
                        agg_t[:],
                        lhsT=g_t[:, blk, :],
                        rhs=o_c[:],
                        start=(ci == 0),
                        stop=(ci == len(blocks) - 1),
                    )

                agg_sb = work.tile([P, P], f16, tag="aggT")
                nc.vector.tensor_copy(agg_sb[:], agg_t[:])
                z_t = psum_b.tile([P, P], f32, space="PSUM")
                nc.tensor.matmul(z_t[:], lhsT=agg_sb[:], rhs=wt_sb[:], start=True, stop=False)
                nc.tensor.matmul(
                    z_t[:], lhsT=s_t[:, t * P : (t + 1) * P], rhs=b_sb[:],
                    start=False, stop=True,
                )

                # elu(z) = max(z, exp(min(z,0)) - 1)
                zmin = work.tile([P, P], f16, tag="zmin")
                nc.vector.tensor_scalar_min(zmin[:], z_t[:], 0.0)
                ez = work.tile([P, P], f16, tag="ez")
                nc.scalar.activation(ez[:], zmin[:], mybir.ActivationFunctionType.Exp)
                em1 = work.tile([P, P], f16, tag="em1")
                nc.vector.tensor_scalar_add(em1[:], ez[:], -1.0)
                nc.vector.tensor_tensor(
                    out=res_t[:, t * D : (t + 1) * D],
                    in0=z_t[:],
                    in1=em1[:],
                    op=mybir.AluOpType.max,
                )
                nc.sync.dma_start(
                    out_d[tg * P : (tg + 1) * P, :], res_t[:, t * D : (t + 1) * D]
                )

    nc.compile()
    return nc


def _prep_inputs(x, W, b, adj_rows, adj_cols, adj_vals):
    """Host-side edge bucketing: group edges by (core, dest tile, src window),
    pad each bucket to 128-edge blocks with block counts shared across cores,
    and lay out per-group gather indices / one-hot scalars."""
    rows = np.ascontiguousarray(adj_rows).astype(np.int64)
    cols = np.ascontiguousarray(adj_cols).astype(np.int64)
    vals = np.ascontiguousarray(adj_vals).astype(np.float32)

    core = rows // RPC
    local = rows - core * RPC
    tloc = local // P
    rloc = (local % P).astype(np.float32)
    gtile = core * TILES + tloc
    w = cols // WND

    key = gtile * NWIN + w
    order = np.argsort(key, kind="stable")
    k_s = key[order]
    c_s = cols[order]
    v_s = vals[order]
    r_s = rloc[order]

    nbuckets = NCORES * TILES * NWIN
    cnt = np.bincount(k_s, minlength=nbuckets).reshape(NCORES, TILES, NWIN)
    nblk_arr = (cnt.max(axis=0) + P - 1) // P          # [TILES, NWIN] shared
    nblk = [[int(nblk_arr[t, wi]) for wi in range(NWIN)] for t in range(TILES)]

    # per-group layout: for w: for t in group: nblk[t][w] blocks
    # bucket (t, w) starts at block blkoff[t][w] within group grp = t // GT
    blkoff = np.zeros((TILES, NWIN), np.int64)
    gblk = np.zeros(NGRP, np.int64)
    for grp in range(NGRP):
        off = 0
        for wi in range(NWIN):
            for t in range(grp * GT, (grp + 1) * GT):
                blkoff[t, wi] = off
                off += nblk_arr[t, wi]
        gblk[grp] = off
    nblk_max = int(gblk.max())
    NB = nblk_max

    # slot index within each core's flat [NGRP, NB, 128] space
    slots_per_bucket = nblk_arr * P                     # [TILES, NWIN]
    # base slot of bucket (t, w) = (t//GT)*NB*128 + blkoff*128
    bucket_base = (np.arange(TILES)[:, None] // GT) * (NB * P) + blkoff * P

    starts = np.zeros(nbuckets, np.int64)
    starts[1:] = np.cumsum(cnt.reshape(-1))[:-1]
    pos = np.arange(len(k_s), dtype=np.int64) - starts[k_s]

    core_s = k_s // (TILES * NWIN)
    tw_s = k_s % (TILES * NWIN)
    t_glob = tw_s // NWIN
    w_glob = tw_s % NWIN
    slot = bucket_base[t_glob, w_glob] + pos            # within-core slot
    dest = core_s * (NGRP * NB * P) + slot

    tot = NCORES * NGRP * NB * P
    idx_pad = np.zeros(tot, np.int16)
    val_pad = np.zeros(tot, np.float32)
    row_pad = np.zeros(tot, np.float32)
    idx_pad[dest] = (c_s - w_glob * WND).astype(np.int16)
    val_pad[dest] = v_s
    row_pad[dest] = r_s

    # idx wrap: slot i of a group -> [16*(k) + i%16, i//16] replicated k=0..7
    idx_g = idx_pad.reshape(NCORES, NGRP, NB, 8, 16)    # i = blk*128 + g*16 + r
    idx_g = idx_g.transpose(0, 1, 4, 2, 3).reshape(NCORES, NGRP, 16, NB * 8)
    idx_full = np.tile(idx_g, (1, 1, 8, 1))             # [NCORES, NGRP, 128, NB*8]

    def bshape(a):
        # [NCORES, NGRP, NB, P] -> [NCORES, NGRP, P, NB]
        return np.ascontiguousarray(
            a.reshape(NCORES, NGRP, NB, P).transpose(0, 1, 3, 2)
        )

    rows_a = bshape(row_pad)
    vals_a = bshape(val_pad)
    rv = np.concatenate([rows_a, vals_a], axis=3)       # [NCORES, NGRP, P, 2NB]

    s_full = np.bincount(
        np.ascontiguousarray(adj_rows), weights=adj_vals.astype(np.float64),
        minlength=N_NODES,
    ).astype(np.float32)
    s_pad = np.zeros(NCORES * RPAD, dtype=np.float32)
    s_pad.reshape(NCORES, RPAD)[:, :RPC] = s_full.reshape(NCORES, RPC)
    s_a = s_pad.reshape(NCORES, NGRP, 1, GT * P).astype(np.float16)

    x_g = np.ascontiguousarray(x).astype(np.float16)
    wt_g = np.ascontiguousarray(W.T).astype(np.float16)
    b_g = np.ascontiguousarray(b).reshape(1, D).astype(np.float16)

    in_maps = []
    for i in range(NCORES):
        in_maps.append(
            {
                "x": x_g,
                "wT": wt_g,
                "bvec": b_g,
                "idx": np.ascontiguousarray(idx_full[i]),
                "rv": np.ascontiguousarray(rv[i]),
                "svec": s_a[i],
            }
        )
    return in_maps, nblk, nblk_max


_CACHE = {}


def _run(in_maps, nblk, nblk_max, trace=False):
    from concourse.bass_utils import run_bass_kernel_spmd

    key = (tuple(map(tuple, nblk)), nblk_max)
    if key not in _CACHE:
        _CACHE[key] = _build_program(nblk, nblk_max)
    nc = _CACHE[key]
    return run_bass_kernel_spmd(nc, in_maps, list(range(NCORES)), trace=trace)


def kernel(x, W, b, adj_rows, adj_cols, adj_vals, trace=False, _return_raw=False):
    x = np.asarray(x)
    in_maps, nblk, nblk_max = _prep_inputs(
        x, np.asarray(W), np.asarray(b), np.asarray(adj_rows),
        np.asarray(adj_cols), np.asarray(adj_vals),
    )
    res = _run(in_maps, nblk, nblk_max, trace=trace)
    outs = [res.results[i]["out"][:RPC] for i in range(NCORES)]
    full = np.concatenate(outs, axis=0).astype(np.float32)
    if _return_raw:
        return full, res
    return full
